# revision 1
# baseline (speedup 1.0000x reference)
"""Trainium2 Bass kernel for nn_NeuralNetworkDPD (dense_mlp).

Strategy (feature-major, 2-token-halves packed on 128 partitions):
  - Each core handles 4 batch rows. A-half = rows {0,1}, B-half = rows {2,3},
    packed as SBUF partitions [0:64)=A-token features, [64:128)=B-token feats.
  - Dense layers: block-diag(W, W) stationary [128,128]; each streamed column
    carries 2 tokens -> 0.5 PE cycles/token/layer.
  - LayerNorm stats as broadcast PLANES: a block-diag(ones/64) stationary
    reduces over the feature partitions and replicates the result to all 64
    output partitions of each half, so mean/var arrive already broadcast:
        mu_bc  = onesd @ z      (one matmul)
        var_bc = onesd @ (z-mu_bc)^2
  - Normalize: v=(z-mu_bc); rs=recip_approx(sqrt(var_bc+eps)); u=Prelu on
    ScalarE fusing gamma (scale), beta (bias), alpha - all per-partition.
  - skip connection and b_out applied host-side (cheap rank-1/elementwise).
"""

import sys
from contextlib import ExitStack

sys.path.insert(0, "/opt/trn_rl_repo")

import numpy as np

import concourse.bacc as bacc
import concourse.bass as bass
import concourse.tile as tile
from concourse import mybir

F = 64          # feature width
NL = 6          # chained dense layers
EPS = 1e-3
CH = 512        # tokens per matmul (PSUM bank)
SUP = 8         # chunks per super-chunk (scheduling window)
R = mybir.dt.float32r   # dtype of all matmul-feeding tensors (1 cyc/row)


def build_kernel(tc, outs, ins, tokens_per_row):
    """Emit the Tile program. ins/outs are dicts of DRAM APs."""
    nc = tc.nc
    TPR = tokens_per_row
    cpr = TPR // CH              # chunks per row
    spr = cpr // SUP             # super-chunks per row
    assert cpr % SUP == 0
    NG = SUP // 2                # groups (of 2 chunks) per super

    xr, xi = ins["xr"], ins["xi"]
    out = outs["out"]            # [4, TPR, 2] fp32

    # Internal padded copies of x: [4, TPR+3], first 3 entries zero.
    xpad_r = nc.dram_tensor("xpad_r", [4, TPR + 3], R,
                            kind="Internal").ap()
    xpad_i = nc.dram_tensor("xpad_i", [4, TPR + 3], R,
                            kind="Internal").ap()

    ctx = ExitStack()
    singles = ctx.enter_context(tc.tile_pool(name="singles", bufs=1))
    zpool = ctx.enter_context(tc.tile_pool(name="zpool", bufs=8))
    rpool = ctx.enter_context(tc.tile_pool(name="rpool", bufs=8))
    upool = ctx.enter_context(tc.tile_pool(name="upool", bufs=3))
    vpool = ctx.enter_context(tc.tile_pool(name="vpool", bufs=4))
    qpool = ctx.enter_context(tc.tile_pool(name="qpool", bufs=4))
    fpool = ctx.enter_context(tc.tile_pool(name="fpool", bufs=4))
    opool = ctx.enter_context(tc.tile_pool(name="opool", bufs=3))
    zp_pool = ctx.enter_context(tc.tile_pool(name="zp", bufs=2, space="PSUM"))
    mu_pool = ctx.enter_context(tc.tile_pool(name="mu", bufs=2, space="PSUM"))
    va_pool = ctx.enter_context(tc.tile_pool(name="va", bufs=2, space="PSUM"))

    # ---- load weights/constants into SBUF ----
    wd = singles.tile([128, NL * 128], R)
    win = singles.tile([16, 128], R)
    wout = singles.tile([128, 4], R)
    onesd = singles.tile([128, 128], R)
    percol = singles.tile([128, 25], mybir.dt.float32)
    epsc = singles.tile([128, 1], mybir.dt.float32)
    nc.sync.dma_start(out=wd, in_=ins["wd"])
    nc.sync.dma_start(out=win, in_=ins["win"])
    nc.sync.dma_start(out=wout, in_=ins["wout"])
    nc.sync.dma_start(out=onesd, in_=ins["onesd"])
    nc.sync.dma_start(out=percol, in_=ins["percol"])
    nc.vector.memset(epsc, EPS)

    b_in_col = percol[:, 0:1]
    dense_b_col = [percol[:, 1 + l: 2 + l] for l in range(NL)]
    gamma_col = [percol[:, 7 + l: 8 + l] for l in range(NL)]
    beta_col = [percol[:, 13 + l: 14 + l] for l in range(NL)]
    alpha_col = [percol[:, 19 + l: 20 + l] for l in range(NL)]

    # ---- build zero-padded x in DRAM ----
    zrow = singles.tile([1, 4], R)
    nc.vector.memset(zrow.bitcast(mybir.dt.float32), 0.0)
    for r in range(4):
        for xp in (xpad_r, xpad_i):
            nc.sync.dma_start(out=xp[r: r + 1, 0:3], in_=zrow[0:1, 0:3])
    nc.sync.dma_start(out=xpad_r[:, 3:], in_=xr)
    nc.sync.dma_start(out=xpad_i[:, 3:], in_=xi)

    # ---------------- main loops ----------------
    for rp in range(2):                     # row-pair: A=row rp, B=row 2+rp
        rowA, rowB = rp, 2 + rp
        for s in range(spr):                # super-chunk
            # -- w_in: windowed feats + first dense for 8 chunks --
            zps = []                        # psum tiles holding current z
            for k in range(SUP):
                t0 = (s * SUP + k) * CH
                feats = fpool.tile([16, CH], R, tag="feats")
                # A-half lags: rows 0-3 real, 4-7 imag; B-half: rows 8-15
                for (base, row) in ((0, rowA), (8, rowB)):
                    src_r = bass.AP(tensor=xpad_r.tensor,
                                    offset=row * (TPR + 3) + t0,
                                    ap=[[1, 4], [1, CH]])
                    src_i = bass.AP(tensor=xpad_i.tensor,
                                    offset=row * (TPR + 3) + t0,
                                    ap=[[1, 4], [1, CH]])
                    nc.sync.dma_start(out=feats[base: base + 4, :], in_=src_r)
                    nc.sync.dma_start(out=feats[base + 4: base + 8, :], in_=src_i)
                if k % 2 == 0:
                    zp = zp_pool.tile([128, 2 * CH], mybir.dt.float32, tag="zp")
                    zps.append(zp)
                nc.tensor.matmul(out=zps[-1][:, (k % 2) * CH:(k % 2 + 1) * CH],
                                 lhsT=(win[:, :]), rhs=(feats),
                                 start=True, stop=True)

            res = [None, None, None]        # z0, z2, z4 anchor groups
            z_groups = [None] * NG

            for l in range(NL + 1):         # 6 LN+PReLU+dense stages + final
                bias = b_in_col if l == 0 else dense_b_col[l - 1]
                new_z = [None] * NG
                for g in range(NG):
                    if l in (0, 2, 4):
                        zt = rpool.tile([128, 2 * CH], R,
                                        tag="za", name=f"za{l}g{g}")
                    else:
                        zt = zpool.tile([128, 2 * CH], R,
                                        tag="z", name=f"z{l}g{g}")
                    nc.scalar.activation(out=zt, in_=zps[g],
                                         func=mybir.ActivationFunctionType.Identity,
                                         bias=bias, scale=1.0)
                    if l in (2, 4, 6):      # residual add at block boundaries
                        if l == 6:
                            zsum = zpool.tile([128, 2 * CH], R,
                                              tag="z", name=f"zs{l}g{g}")
                        else:
                            zsum = rpool.tile([128, 2 * CH], R,
                                              tag="zb", name=f"zs{l}g{g}")
                        nc.vector.tensor_add(zsum, zt, res[l // 2 - 1][g])
                        zt = zsum
                    new_z[g] = zt
                z_groups = new_z
                if l in (0, 2, 4):
                    res[l // 2] = z_groups
                if l == NL:
                    break

                zps = []
                for g in range(NG):
                    zg = z_groups[g]
                    u = upool.tile([128, 2 * CH], R, tag="u")
                    for j in range(2):
                        zsl = zg[:, j * CH:(j + 1) * CH]
                        # mean plane (already broadcast to both halves)
                        mu = mu_pool.tile([128, CH], mybir.dt.float32, tag="mu")
                        nc.tensor.matmul(out=mu, lhsT=(onesd[:, :]),
                                         rhs=(zsl), start=True, stop=True)
                        v = vpool.tile([128, CH], mybir.dt.float32, tag="v")
                        nc.vector.tensor_sub(v, zsl, mu)
                        vsq = qpool.tile([128, CH], R, tag="vsq")
                        nc.scalar.activation(
                            out=vsq, in_=v,
                            func=mybir.ActivationFunctionType.Square)
                        va = va_pool.tile([128, CH], mybir.dt.float32, tag="va")
                        nc.tensor.matmul(out=va, lhsT=(onesd[:, :]),
                                         rhs=(vsq), start=True, stop=True)
                        sg = qpool.tile([128, CH], mybir.dt.float32, tag="sg")
                        nc.scalar.activation(
                            out=sg, in_=va,
                            func=mybir.ActivationFunctionType.Sqrt,
                            bias=epsc, scale=1.0)
                        rs = vpool.tile([128, CH], mybir.dt.float32, tag="rs")
                        nc.vector.reciprocal_approx_fast(out=rs, in_=sg)
                        nc.vector.tensor_mul(u[:, j * CH:(j + 1) * CH], v, rs)
                    # PReLU(gamma*x + beta) fused on ScalarE, in place on u
                    nc.scalar.activation(out=u, in_=u,
                                         func=mybir.ActivationFunctionType.Prelu,
                                         bias=beta_col[l], scale=gamma_col[l],
                                         alpha=alpha_col[l])
                    zp = zp_pool.tile([128, 2 * CH], mybir.dt.float32, tag="zp")
                    for j in range(2):
                        nc.tensor.matmul(
                            out=zp[:, j * CH:(j + 1) * CH],
                            lhsT=(wd[:, l * 128:(l + 1) * 128]),
                            rhs=(u[:, j * CH:(j + 1) * CH]),
                            start=True, stop=True)
                    zps.append(zp)

            # -- w_out + store --
            for g in range(NG):
                for j in range(2):
                    k = 2 * g + j
                    t0 = (s * SUP + k) * CH
                    op = mu_pool.tile([4, CH], mybir.dt.float32, tag="mu",
                                      padded_shape=[128, CH])
                    nc.tensor.matmul(out=op, lhsT=(wout[:, :]),
                                     rhs=(z_groups[g][:, j * CH:(j + 1) * CH]),
                                     start=True, stop=True)
                    ot = opool.tile([4, CH], mybir.dt.float32, tag="ot")
                    nc.scalar.copy(out=ot, in_=op)
                    for (half, row) in ((0, rowA), (1, rowB)):
                        dst = bass.AP(tensor=out.tensor,
                                      offset=row * TPR * 2 + t0 * 2,
                                      ap=[[1, 2], [2, CH]])
                        nc.sync.dma_start(out=dst,
                                          in_=ot[2 * half: 2 * half + 2, :])
    ctx.close()


def _host_pack(inputs):
    """Build the shared (replicated) packed-weight arrays."""
    w_in = np.asarray(inputs["w_in"], np.float32)
    dense_w = np.asarray(inputs["dense_w"], np.float32)
    w_out = np.asarray(inputs["w_out"], np.float32)
    ln_gamma = np.asarray(inputs["ln_gamma"], np.float32)
    ln_beta = np.asarray(inputs["ln_beta"], np.float32)
    alpha = np.asarray(inputs["alpha"], np.float32)
    b_in = np.asarray(inputs["b_in"], np.float32)
    dense_b = np.asarray(inputs["dense_b"], np.float32)

    wd = np.zeros((128, NL * 128), np.float32)
    for l in range(NL):
        wd[0:64, l * 128: l * 128 + 64] = dense_w[l]
        wd[64:128, l * 128 + 64: l * 128 + 128] = dense_w[l]
    win = np.zeros((16, 128), np.float32)
    win[0:8, 0:64] = w_in
    win[8:16, 64:128] = w_in
    wout = np.zeros((128, 4), np.float32)
    wout[0:64, 0:2] = w_out
    wout[64:128, 2:4] = w_out
    onesd = np.zeros((128, 128), np.float32)
    onesd[0:64, 0:64] = 1.0 / F
    onesd[64:128, 64:128] = 1.0 / F
    percol = np.zeros((128, 25), np.float32)
    percol[:, 0] = np.tile(b_in, 2)
    for l in range(NL):
        percol[:, 1 + l] = np.tile(dense_b[l], 2)
        percol[:, 7 + l] = np.tile(ln_gamma[l], 2)
        percol[:, 13 + l] = np.tile(ln_beta[l], 2)
        percol[:, 19 + l] = np.tile(alpha[l], 2)
    return dict(wd=wd, win=win, wout=wout, onesd=onesd, percol=percol)


def build_program(tokens_per_row):
    """Build the full Bass/Tile program for one core's shard."""
    nc = bacc.Bacc("TRN2")
    ins = {}
    shapes = dict(wd=(128, NL * 128), win=(16, 128), wout=(128, 4),
                  onesd=(128, 128), percol=(128, 25))
    for name, shp in shapes.items():
        dt = mybir.dt.float32 if name == "percol" else R
        ins[name] = nc.dram_tensor(name, list(shp), dt,
                                   kind="ExternalInput").ap()
    ins["xr"] = nc.dram_tensor("xr", [4, tokens_per_row], R,
                               kind="ExternalInput").ap()
    ins["xi"] = nc.dram_tensor("xi", [4, tokens_per_row], R,
                               kind="ExternalInput").ap()
    outs = {"out": nc.dram_tensor("out", [4, tokens_per_row, 2],
                                  mybir.dt.float32, kind="ExternalOutput").ap()}
    with tile.TileContext(nc) as tc:
        build_kernel(tc, outs, ins, tokens_per_row)
    nc.compile()
    return nc


def _run(inputs, trace=False):
    from concourse.bass_utils import run_bass_kernel_spmd

    x_real = np.asarray(inputs["x_real"], np.float32)
    x_imag = np.asarray(inputs["x_imag"], np.float32)
    B, N = x_real.shape
    n_cores = 8
    rows_per_core = B // n_cores

    shared = _host_pack(inputs)
    nc = build_program(N)

    in_maps = []
    for c in range(n_cores):
        m = dict(shared)
        m["xr"] = np.ascontiguousarray(x_real[c * rows_per_core:(c + 1) * rows_per_core])
        m["xi"] = np.ascontiguousarray(x_imag[c * rows_per_core:(c + 1) * rows_per_core])
        in_maps.append(m)

    res = run_bass_kernel_spmd(nc, in_maps, core_ids=list(range(n_cores)),
                               trace=trace)
    outs_np = [r["out"] for r in res.results]
    full = np.concatenate(outs_np, axis=0)          # [B, N, 2]
    b_out = np.asarray(inputs["b_out"], np.float32)
    re = full[..., 0] + b_out[0] + x_real
    im = full[..., 1] + b_out[1] + x_imag
    return (re + 1j * im).astype(np.complex64), res


def kernel(**inputs):
    return _run(inputs, trace=False)[0]



# revision 18
# speedup vs baseline: 1.3604x; 1.3604x over previous
"""Trainium2 Bass kernel for nn_NeuralNetworkDPD (dense_mlp).

Hardware reality (measured): a cross-engine dependency hop costs ~2.3us
(semaphore wake-up), so throughput = hops_per_chunk * 2.3us / pipeline_depth.
The design therefore minimizes PSUM residency so MANY chunks can be in
flight at once (depth ~8-12), hiding the hop latency entirely:

  - Feature-major, 2-token-halves packed on 128 partitions (A-half rows
    {0,1} on partitions 0:64, B-half rows {2,3} on 64:128).
  - Centered carry: every dense stationary is W @ (I - J/64) so matmul
    outputs are pre-centered (LN mean subtraction is free). The final
    per-token mean is recovered with 4 rank-1 streams (feats, p1, p3, p5)
    accumulated into the w_out PSUM.
  - Every PSUM tile has exactly ONE consumer which immediately evacuates
    it to SBUF bf16:  vb = (dense_psum + bias)  [DVE tensor_scalar].
    Residual carry lives in SBUF:  c' = (dense_psum + bias) + c  [DVE stt].
  - Per layer: vb(V) -> vsq=vb*vb(V) -> va=ones' @ vsq(PE) ->
    rs=AbsRsqrt(va + eps/g^2)(S) -> w=vb*rs(V) -> p=Prelu(w+beta;alpha)(S)
    -> dense(PE).  gamma rides the stats stationary (1/(64 g_o^2)) so
    rs = gamma/sigma directly.
"""

import sys
from contextlib import ExitStack

sys.path.insert(0, "/opt/trn_rl_repo")

import numpy as np

import concourse.bacc as bacc
import concourse.bass as bass
import concourse.tile as tile
from concourse import mybir

F = 64          # feature width
NL = 6          # chained dense layers
EPS = 1e-3
CH = 512        # tokens-per-half per chunk (one PSUM bank)
G = 8           # chunks issued stage-blocked (pipeline depth)
BF = mybir.dt.bfloat16
F32 = mybir.dt.float32
ALU = mybir.AluOpType

PRELU_S_MOD = 1        # (chunk+1) % mod == 0 -> PReLU on ScalarE

# percol column layout
BR = 0          # read-bias for vb at k=0,1,3,5      (4 cols: idx by k)
BC = 6          # carry-add bias for k=1,3,5         (cols BC+k)
EG = 12         # eps/gamma_o^2 per layer            (6 cols)
BE = 18         # beta per layer                     (6 cols)
AL = 24         # alpha per layer                    (6 cols)
AB = 30         # alpha*beta per layer               (6 cols)
NPC = 36


def build_kernel(tc, outs, ins, tokens_per_row):
    nc = tc.nc
    TPR = tokens_per_row
    cpr = TPR // CH
    nchunks = 2 * cpr            # two row-pairs
    out = outs["out"]            # [4, TPR, 2] fp32
    xr, xi = ins["xr"], ins["xi"]          # [4, TPR+3] bf16, host-padded

    ctx = ExitStack()
    singles = ctx.enter_context(tc.tile_pool(name="singles", bufs=1))
    fpool = ctx.enter_context(tc.tile_pool(name="fpool", bufs=2 * G + 2))
    vbpool = ctx.enter_context(tc.tile_pool(name="vb", bufs=G + 2))
    vqpool = ctx.enter_context(tc.tile_pool(name="vq", bufs=G + 2))
    rspool = ctx.enter_context(tc.tile_pool(name="rs", bufs=G + 2))
    wpool = ctx.enter_context(tc.tile_pool(name="w", bufs=G + 2))
    qpool = ctx.enter_context(tc.tile_pool(name="q", bufs=G + 2))
    cpool = ctx.enter_context(tc.tile_pool(name="cp", bufs=2 * G + 2))
    ptpool = ctx.enter_context(tc.tile_pool(name="pt", bufs=G + 2))
    pkpool = ctx.enter_context(tc.tile_pool(name="pk", bufs=3 * G + 3))
    otpool = ctx.enter_context(tc.tile_pool(name="ot", bufs=G))
    bpool = ctx.enter_context(tc.tile_pool(name="bp", bufs=4, space="PSUM"))
    vapool = ctx.enter_context(tc.tile_pool(name="va", bufs=4, space="PSUM"))

    # ---- load weights/constants into SBUF ----
    wd = singles.tile([128, NL * 128], BF)
    ones6 = singles.tile([128, NL * 128], BF)
    win = singles.tile([16, 128], BF)
    wtail = singles.tile([128, 4], BF)
    fext = singles.tile([16, 4], BF)
    pext = singles.tile([128, 12], BF)
    percol = singles.tile([128, NPC], F32)
    nc.sync.dma_start(out=wd, in_=ins["wd"])
    nc.sync.dma_start(out=ones6, in_=ins["ones6"])
    nc.sync.dma_start(out=win, in_=ins["win"])
    nc.sync.dma_start(out=wtail, in_=ins["wtail"])
    nc.sync.dma_start(out=fext, in_=ins["fext"])
    nc.sync.dma_start(out=pext, in_=ins["pext"])
    nc.sync.dma_start(out=percol, in_=ins["percol"])

    col = lambda base, k: percol[:, base + k: base + k + 1]

    def chunk_rowt(f):
        rp, ci = f // cpr, f % cpr
        return rp, 2 + rp, ci * CH

    state = {}

    def stage_load(f):
        rowA, rowB, t0 = chunk_rowt(f)
        feats = fpool.tile([16, CH], BF, tag="feats", name=f"feats{f}")
        for (base, row) in ((0, rowA), (8, rowB)):
            for (boff, xp) in ((0, xr), (4, xi)):
                src = bass.AP(tensor=xp.tensor,
                              offset=row * (TPR + 3) + t0,
                              ap=[[1, 4], [1, CH]])
                nc.sync.dma_start(
                    out=feats[base + boff: base + boff + 4, :], in_=src)
        state[f] = {"feats": feats, "pk": {}}

    def stage_win(f):
        st = state[f]
        b = bpool.tile([128, CH], F32, tag="b", name=f"z0_{f}")
        nc.tensor.matmul(out=b, lhsT=win, rhs=st["feats"],
                         start=True, stop=True)
        st["b"] = b

    def stage_vb(f, k):
        """Evacuate the dense/win psum (single reader) or alias the carry."""
        st = state[f]
        if k in (2, 4):
            st["vb"] = st["carry"]          # bias already folded in
            return
        vb = vbpool.tile([128, CH], BF, tag="vb", name=f"vb{f}_{k}")
        nc.vector.tensor_scalar_add(vb, st["b"], col(BR, k))
        st["vb"] = vb
        if k == 0:
            st["carry"] = vb                # c0 = z0 + b_in

    def stage_vsq(f, k):
        st = state[f]
        vsq = vqpool.tile([128, CH], BF, tag="vsq", name=f"vsq{f}_{k}")
        nc.vector.tensor_tensor(out=vsq, in0=st["vb"], in1=st["vb"],
                                op=ALU.mult)
        st["vsq"] = vsq

    def stage_va(f, k):
        st = state[f]
        va = vapool.tile([128, CH], F32, tag="va", name=f"va{f}_{k}")
        nc.tensor.matmul(out=va, lhsT=ones6[:, k * 128:(k + 1) * 128],
                         rhs=st["vsq"], start=True, stop=True)
        st["va"] = va

    def stage_rs(f, k):
        st = state[f]
        rs = rspool.tile([128, CH], BF, tag="rs", name=f"rs{f}_{k}")
        nc.scalar.activation(
            out=rs, in_=st["va"],
            func=mybir.ActivationFunctionType.Abs_reciprocal_sqrt,
            bias=col(EG, k), scale=1.0)
        st["rs"] = rs

    def stage_w(f, k):
        st = state[f]
        w = wpool.tile([128, CH], BF, tag="w", name=f"w{f}_{k}")
        nc.vector.tensor_tensor(out=w, in0=st["vb"], in1=st["rs"],
                                op=ALU.mult)
        st["w"] = w

    def stage_prelu(f, k):
        st = state[f]
        w = st["w"]
        pool = pkpool if k % 2 == 1 else ptpool
        p = pool.tile([128, CH], BF, tag="p", name=f"p{f}_{k}")
        if (f + 1) % PRELU_S_MOD == 0:
            nc.scalar.activation(out=p, in_=w,
                                 func=mybir.ActivationFunctionType.Prelu,
                                 bias=col(BE, k), scale=1.0, alpha=col(AL, k))
        else:
            # p = max(w + beta, alpha*w + alpha*beta); valid for alpha <= 1
            q = qpool.tile([128, CH], BF, tag="q", name=f"q{f}_{k}")
            nc.vector.tensor_scalar(out=q, in0=w, scalar1=col(AL, k),
                                    scalar2=col(AB, k), op0=ALU.mult,
                                    op1=ALU.add)
            t = qpool.tile([128, CH], BF, tag="q", name=f"t{f}_{k}")
            nc.vector.tensor_scalar(out=t, in0=w, scalar1=col(BE, k),
                                    scalar2=None, op0=ALU.add)
            nc.vector.tensor_tensor(out=p, in0=t, in1=q, op=ALU.max)
        if k % 2 == 1:
            st["pk"][k] = p
        st["p"] = p

    def stage_dense(f, k):
        st = state[f]
        b = bpool.tile([128, CH], F32, tag="b", name=f"b{f}_{k}")
        nc.tensor.matmul(out=b, lhsT=wd[:, k * 128:(k + 1) * 128],
                         rhs=st["p"], start=True, stop=True)
        st["b"] = b

    def stage_carry(f, k):
        """After dense k in {1,3,5}: c' = (dense_psum + bc_k) + c."""
        st = state[f]
        c = cpool.tile([128, CH], BF, tag="c", name=f"c{f}_{k}")
        nc.vector.scalar_tensor_tensor(out=c, in0=st["b"], scalar=col(BC, k),
                                       in1=st["carry"], op0=ALU.add,
                                       op1=ALU.add)
        st["carry"] = c

    def tail_opmm(f):
        st = state[f]
        op = vapool.tile([4, CH], F32, tag="va", padded_shape=[128, CH],
                         name=f"op{f}")
        nc.tensor.matmul(out=op, lhsT=wtail, rhs=st["carry"],
                         start=True, stop=False, skip_group_check=True)
        nc.tensor.matmul(out=op, lhsT=fext, rhs=st["feats"],
                         start=False, stop=False, skip_group_check=True)
        for j, k in enumerate((1, 3, 5)):
            nc.tensor.matmul(out=op, lhsT=pext[:, 4 * j: 4 * j + 4],
                             rhs=st["pk"][k], start=False, stop=(k == 5),
                             skip_group_check=True)
        st["op"] = op

    def tail_store(f):
        st = state[f]
        rowA, rowB, t0 = chunk_rowt(f)
        ot = otpool.tile([4, CH], F32, tag="ot", name=f"ot{f}")
        nc.scalar.copy(out=ot, in_=st["op"])
        for (half, row) in ((0, rowA), (1, rowB)):
            dst = bass.AP(tensor=out.tensor,
                          offset=row * TPR * 2 + t0 * 2,
                          ap=[[1, 2], [2, CH]])
            nc.sync.dma_start(out=dst, in_=ot[2 * half: 2 * half + 2, :])
        del state[f]

    def emit_layer(grp, k):
        for f in grp:
            stage_vb(f, k)
        for f in grp:
            stage_vsq(f, k)
        for f in grp:
            stage_va(f, k)
        for f in grp:
            stage_rs(f, k)
        for f in grp:
            stage_w(f, k)
        for f in grp:
            stage_prelu(f, k)
        for f in grp:
            stage_dense(f, k)
        if k in (1, 3, 5):
            for f in grp:
                stage_carry(f, k)

    # ---- main loop: groups of G chunks, tails overlapped with the next
    # group's first layer ----
    groups = [list(range(f0, min(f0 + G, nchunks)))
              for f0 in range(0, nchunks, G)]
    prev = None
    for grp in groups:
        for f in grp:
            stage_load(f)
        for f in grp:
            stage_win(f)
        emit_layer(grp, 0)
        if prev is not None:
            for f in prev:
                tail_opmm(f)
            for f in prev:
                tail_store(f)
        for k in range(1, NL):
            emit_layer(grp, k)
        prev = grp
    for f in prev:
        tail_opmm(f)
    for f in prev:
        tail_store(f)
    ctx.close()


def _host_pack(inputs):
    """Build the shared (replicated) packed-weight arrays."""
    w_in = np.asarray(inputs["w_in"], np.float32)
    dense_w = np.asarray(inputs["dense_w"], np.float32)
    w_out = np.asarray(inputs["w_out"], np.float32)
    ln_gamma = np.asarray(inputs["ln_gamma"], np.float32)
    ln_beta = np.asarray(inputs["ln_beta"], np.float32)
    alpha = np.asarray(inputs["alpha"], np.float32)
    b_in = np.asarray(inputs["b_in"], np.float32)
    dense_b = np.asarray(inputs["dense_b"], np.float32)

    C = np.eye(F, dtype=np.float32) - 1.0 / F   # centering projector

    win = np.zeros((16, 128), np.float32)
    winC = w_in @ C
    win[0:8, 0:64] = winC
    win[8:16, 64:128] = winC

    wd = np.zeros((128, NL * 128), np.float32)
    ones6 = np.zeros((128, NL * 128), np.float32)
    for l in range(NL):
        wdC = dense_w[l] @ C
        wd[0:64, l * 128: l * 128 + 64] = wdC
        wd[64:128, l * 128 + 64: l * 128 + 128] = wdC
        g2 = ln_gamma[l] ** 2                     # [F]
        blk = np.repeat((1.0 / (F * g2))[None, :], F, axis=0)  # [F_in, F_out]
        ones6[0:64, l * 128: l * 128 + 64] = blk
        ones6[64:128, l * 128 + 64: l * 128 + 128] = blk

    wtail = np.zeros((128, 4), np.float32)
    wtail[0:64, 0:2] = w_out
    wtail[64:128, 2:4] = w_out

    s = w_out.sum(axis=0)                         # [2]
    wbar_in = w_in.mean(axis=1)                   # [8]
    fext = np.zeros((16, 4), np.float32)
    fext[0:8, 0:2] = np.outer(wbar_in, s)
    fext[8:16, 2:4] = np.outer(wbar_in, s)
    pext = np.zeros((128, 12), np.float32)
    for j, l in enumerate((1, 3, 5)):
        wbar = dense_w[l].mean(axis=1)            # [F]
        pext[0:64, 4 * j: 4 * j + 2] = np.outer(wbar, s)
        pext[64:128, 4 * j + 2: 4 * j + 4] = np.outer(wbar, s)

    # biases (centered-carry bookkeeping)
    bc_in = C @ b_in
    bc = [C @ dense_b[l] for l in range(NL)]
    vb_bias = {0: bc_in, 1: bc[0], 3: bc[2], 5: bc[4]}

    percol = np.zeros((128, NPC), np.float32)
    for k, v in vb_bias.items():
        percol[:, BR + k] = np.tile(v, 2)
    for k in (1, 3, 5):
        percol[:, BC + k] = np.tile(bc[k], 2)
    for k in range(NL):
        g = ln_gamma[k]
        percol[:, EG + k] = np.tile(EPS / (g * g), 2)
        percol[:, BE + k] = np.tile(ln_beta[k], 2)
        percol[:, AL + k] = np.tile(alpha[k], 2)
        percol[:, AB + k] = np.tile(alpha[k] * ln_beta[k], 2)

    # constant part of the final mean correction, folded into b_out
    m_const = b_in.mean() + dense_b[1].mean() + dense_b[3].mean() \
        + dense_b[5].mean()
    b_out_eff = np.asarray(inputs["b_out"], np.float32) + m_const * s

    bf_np = mybir.dt.np(BF)
    shared = dict(wd=wd, ones6=ones6, win=win, wtail=wtail, fext=fext,
                  pext=pext)
    shared = {k: np.ascontiguousarray(v.astype(bf_np))
              for k, v in shared.items()}
    shared["percol"] = percol
    return shared, b_out_eff


def _prep_x(x, dtype):
    """Left-pad 3 zeros per row and convert to bf16."""
    x = np.asarray(x, np.float32)
    xp = np.pad(x, ((0, 0), (3, 0)))
    return np.ascontiguousarray(xp.astype(dtype))


def build_program(tokens_per_row):
    """Build the full Bass/Tile program for one core's shard."""
    nc = bacc.Bacc("TRN2")
    ins = {}
    shapes = dict(wd=(128, NL * 128), ones6=(128, NL * 128), win=(16, 128),
                  wtail=(128, 4), fext=(16, 4), pext=(128, 12),
                  percol=(128, NPC))
    for name, shp in shapes.items():
        dt = F32 if name == "percol" else BF
        ins[name] = nc.dram_tensor(name, list(shp), dt,
                                   kind="ExternalInput").ap()
    ins["xr"] = nc.dram_tensor("xr", [4, tokens_per_row + 3], BF,
                               kind="ExternalInput").ap()
    ins["xi"] = nc.dram_tensor("xi", [4, tokens_per_row + 3], BF,
                               kind="ExternalInput").ap()
    outs = {"out": nc.dram_tensor("out", [4, tokens_per_row, 2],
                                  F32, kind="ExternalOutput").ap()}
    with tile.TileContext(nc) as tc:
        build_kernel(tc, outs, ins, tokens_per_row)
    nc.compile()
    return nc


def _run(inputs, trace=False):
    from concourse.bass_utils import run_bass_kernel_spmd

    x_real = np.asarray(inputs["x_real"], np.float32)
    x_imag = np.asarray(inputs["x_imag"], np.float32)
    B, N = x_real.shape
    n_cores = 8
    rows_per_core = B // n_cores

    shared, b_out_eff = _host_pack(inputs)
    nc = build_program(N)
    bf_np = mybir.dt.np(BF)

    in_maps = []
    for c in range(n_cores):
        m = dict(shared)
        sl = slice(c * rows_per_core, (c + 1) * rows_per_core)
        m["xr"] = _prep_x(x_real[sl], bf_np)
        m["xi"] = _prep_x(x_imag[sl], bf_np)
        in_maps.append(m)

    res = run_bass_kernel_spmd(nc, in_maps, core_ids=list(range(n_cores)),
                               trace=trace)
    outs_np = [r["out"] for r in res.results]
    full = np.concatenate(outs_np, axis=0)          # [B, N, 2]
    re = full[..., 0] + b_out_eff[0] + x_real
    im = full[..., 1] + b_out_eff[1] + x_imag
    return (re + 1j * im).astype(np.complex64), res


def kernel(**inputs):
    return _run(inputs, trace=False)[0]


# revision 27
# speedup vs baseline: 3.0719x; 2.2581x over previous
"""Trainium2 Bass kernel for nn_NeuralNetworkDPD (dense_mlp).

Hardware reality (measured): a cross-engine dependency hop costs ~2.3us
(semaphore wake-up), so throughput = hops_per_chunk * 2.3us / pipeline_depth.
The design therefore minimizes PSUM residency so MANY chunks can be in
flight at once (depth ~8-12), hiding the hop latency entirely:

  - Feature-major, 2-token-halves packed on 128 partitions (A-half rows
    {0,1} on partitions 0:64, B-half rows {2,3} on 64:128).
  - Centered carry: every dense stationary is W @ (I - J/64) so matmul
    outputs are pre-centered (LN mean subtraction is free). The final
    per-token mean is recovered with 4 rank-1 streams (feats, p1, p3, p5)
    accumulated into the w_out PSUM.
  - Every PSUM tile has exactly ONE consumer which immediately evacuates
    it to SBUF bf16:  vb = (dense_psum + bias)  [DVE tensor_scalar].
    Residual carry lives in SBUF:  c' = (dense_psum + bias) + c  [DVE stt].
  - Per layer: vb(V) -> vsq=vb*vb(V) -> va=ones' @ vsq(PE) ->
    rs=AbsRsqrt(va + eps/g^2)(S) -> w=vb*rs(V) -> p=Prelu(w+beta;alpha)(S)
    -> dense(PE).  gamma rides the stats stationary (1/(64 g_o^2)) so
    rs = gamma/sigma directly.
"""

import sys
from contextlib import ExitStack

sys.path.insert(0, "/opt/trn_rl_repo")

import numpy as np

import concourse.bacc as bacc
import concourse.bass as bass
import concourse.tile as tile
from concourse import mybir

F = 64          # feature width
NL = 6          # chained dense layers
EPS = 1e-3
CH = 512        # tokens-per-half per chunk (one PSUM bank)
G = 8           # chunks issued stage-blocked (pipeline depth)
BF = mybir.dt.bfloat16
F32 = mybir.dt.float32
ALU = mybir.AluOpType

PRELU_S_MOD = 1        # (chunk+1) % mod == 0 -> PReLU on ScalarE

# percol column layout
BR = 0          # read-bias for vb at k=0,1,3,5      (4 cols: idx by k)
BC = 6          # carry-add bias for k=1,3,5         (cols BC+k)
EG = 12         # eps/gamma_o^2 per layer            (6 cols)
BE = 18         # beta per layer                     (6 cols)
AL = 24         # alpha per layer                    (6 cols)
AB = 30         # alpha*beta per layer               (6 cols)
NPC = 36


def build_kernel(tc, outs, ins, tokens_per_row):
    nc = tc.nc
    TPR = tokens_per_row
    cpr = TPR // CH
    nchunks = 2 * cpr            # two row-pairs
    out = outs["out"]            # [4, 2, TPR] fp32 (planar re/im)
    xp = ins["xp"]               # [4, 2, TPR+3] bf16, host-padded (re/im)

    ctx = ExitStack()
    singles = ctx.enter_context(tc.tile_pool(name="singles", bufs=1))
    fpool = ctx.enter_context(tc.tile_pool(name="fpool", bufs=2 * G + 2))
    vbpool = ctx.enter_context(tc.tile_pool(name="vb", bufs=G + 2))
    vqpool = ctx.enter_context(tc.tile_pool(name="vq", bufs=G + 2))
    rspool = ctx.enter_context(tc.tile_pool(name="rs", bufs=G + 2))
    wpool = ctx.enter_context(tc.tile_pool(name="w", bufs=G + 2))
    qpool = ctx.enter_context(tc.tile_pool(name="q", bufs=G + 2))
    cpool = ctx.enter_context(tc.tile_pool(name="cp", bufs=2 * G + 2))
    ptpool = ctx.enter_context(tc.tile_pool(name="pt", bufs=G + 2))
    pkpool = ctx.enter_context(tc.tile_pool(name="pk", bufs=3 * G + 3))
    otpool = ctx.enter_context(tc.tile_pool(name="ot", bufs=G))
    bpool = ctx.enter_context(tc.tile_pool(name="bp", bufs=4, space="PSUM"))
    vapool = ctx.enter_context(tc.tile_pool(name="va", bufs=4, space="PSUM"))

    # ---- load weights/constants into SBUF ----
    wd = singles.tile([128, NL * 128], BF)
    ones6 = singles.tile([128, NL * 128], BF)
    win = singles.tile([16, 128], BF)
    wtail = singles.tile([128, 4], BF)
    fext = singles.tile([16, 4], BF)
    pext = singles.tile([128, 12], BF)
    percol = singles.tile([128, NPC], F32)
    nc.sync.dma_start(out=wd, in_=ins["wd"])
    nc.sync.dma_start(out=ones6, in_=ins["ones6"])
    nc.sync.dma_start(out=win, in_=ins["win"])
    nc.sync.dma_start(out=wtail, in_=ins["wtail"])
    nc.sync.dma_start(out=fext, in_=ins["fext"])
    nc.sync.dma_start(out=pext, in_=ins["pext"])
    nc.sync.dma_start(out=percol, in_=ins["percol"])

    col = lambda base, k: percol[:, base + k: base + k + 1]

    def chunk_rowt(f):
        rp, ci = f // cpr, f % cpr
        return rp, 2 + rp, ci * CH

    state = {}

    def stage_load(f):
        rowA, rowB, t0 = chunk_rowt(f)
        T = TPR + 3
        feats = fpool.tile([16, CH], BF, tag="feats", name=f"feats{f}")
        # per half: partitions (re/im:2) x (lag:4); free: CH columns
        for h, row in ((0, rowA), (1, rowB)):
            src = bass.AP(tensor=xp.tensor,
                          offset=row * 2 * T + t0,
                          ap=[[T, 2], [1, 4], [1, CH]])
            nc.sync.dma_start(out=feats[8 * h: 8 * h + 8, :], in_=src)
        state[f] = {"feats": feats, "pk": {}}

    def stage_win(f):
        st = state[f]
        b = bpool.tile([128, CH], F32, tag="b", name=f"z0_{f}")
        nc.tensor.matmul(out=b, lhsT=win, rhs=st["feats"],
                         start=True, stop=True)
        st["b"] = b

    def stage_vb(f, k):
        """Evacuate the dense/win psum (single reader) or alias the carry."""
        st = state[f]
        if k in (2, 4):
            st["vb"] = st["carry"]          # bias already folded in
            return
        vb = vbpool.tile([128, CH], BF, tag="vb", name=f"vb{f}_{k}")
        nc.vector.tensor_scalar_add(vb, st["b"], col(BR, k))
        st["vb"] = vb
        if k == 0:
            st["carry"] = vb                # c0 = z0 + b_in

    def stage_vsq(f, k):
        st = state[f]
        vsq = vqpool.tile([128, CH], BF, tag="vsq", name=f"vsq{f}_{k}")
        nc.vector.tensor_tensor(out=vsq, in0=st["vb"], in1=st["vb"],
                                op=ALU.mult)
        st["vsq"] = vsq

    def stage_va(f, k):
        st = state[f]
        va = vapool.tile([128, CH], F32, tag="va", name=f"va{f}_{k}")
        nc.tensor.matmul(out=va, lhsT=ones6[:, k * 128:(k + 1) * 128],
                         rhs=st["vsq"], start=True, stop=True)
        st["va"] = va

    def stage_rs(f, k):
        st = state[f]
        rs = rspool.tile([128, CH], BF, tag="rs", name=f"rs{f}_{k}")
        nc.scalar.activation(
            out=rs, in_=st["va"],
            func=mybir.ActivationFunctionType.Abs_reciprocal_sqrt,
            bias=col(EG, k), scale=1.0)
        st["rs"] = rs

    def stage_w(f, k):
        st = state[f]
        w = wpool.tile([128, CH], BF, tag="w", name=f"w{f}_{k}")
        nc.vector.tensor_tensor(out=w, in0=st["vb"], in1=st["rs"],
                                op=ALU.mult)
        st["w"] = w

    def stage_prelu(f, k):
        st = state[f]
        w = st["w"]
        pool = pkpool if k % 2 == 1 else ptpool
        p = pool.tile([128, CH], BF, tag="p", name=f"p{f}_{k}")
        if (f + 1) % PRELU_S_MOD == 0:
            nc.scalar.activation(out=p, in_=w,
                                 func=mybir.ActivationFunctionType.Prelu,
                                 bias=col(BE, k), scale=1.0, alpha=col(AL, k))
        else:
            # p = max(w + beta, alpha*w + alpha*beta); valid for alpha <= 1
            q = qpool.tile([128, CH], BF, tag="q", name=f"q{f}_{k}")
            nc.vector.tensor_scalar(out=q, in0=w, scalar1=col(AL, k),
                                    scalar2=col(AB, k), op0=ALU.mult,
                                    op1=ALU.add)
            t = qpool.tile([128, CH], BF, tag="q", name=f"t{f}_{k}")
            nc.vector.tensor_scalar(out=t, in0=w, scalar1=col(BE, k),
                                    scalar2=None, op0=ALU.add)
            nc.vector.tensor_tensor(out=p, in0=t, in1=q, op=ALU.max)
        if k % 2 == 1:
            st["pk"][k] = p
        st["p"] = p

    def stage_dense(f, k):
        st = state[f]
        b = bpool.tile([128, CH], F32, tag="b", name=f"b{f}_{k}")
        nc.tensor.matmul(out=b, lhsT=wd[:, k * 128:(k + 1) * 128],
                         rhs=st["p"], start=True, stop=True)
        st["b"] = b

    def stage_carry(f, k):
        """After dense k in {1,3,5}: c' = (dense_psum + bc_k) + c."""
        st = state[f]
        c = cpool.tile([128, CH], BF, tag="c", name=f"c{f}_{k}")
        nc.vector.scalar_tensor_tensor(out=c, in0=st["b"], scalar=col(BC, k),
                                       in1=st["carry"], op0=ALU.add,
                                       op1=ALU.add)
        st["carry"] = c

    def tail_opmm(f):
        st = state[f]
        op = vapool.tile([4, CH], F32, tag="va", padded_shape=[128, CH],
                         name=f"op{f}")
        nc.tensor.matmul(out=op, lhsT=wtail, rhs=st["carry"],
                         start=True, stop=False, skip_group_check=True)
        nc.tensor.matmul(out=op, lhsT=fext, rhs=st["feats"],
                         start=False, stop=False, skip_group_check=True)
        for j, k in enumerate((1, 3, 5)):
            nc.tensor.matmul(out=op, lhsT=pext[:, 4 * j: 4 * j + 4],
                             rhs=st["pk"][k], start=False, stop=(k == 5),
                             skip_group_check=True)
        st["op"] = op

    def tail_store(f):
        st = state[f]
        rowA, rowB, t0 = chunk_rowt(f)
        ot = otpool.tile([4, CH], F32, tag="ot", name=f"ot{f}")
        nc.scalar.copy(out=ot, in_=st["op"])
        # out is planar [4, 2, TPR]; partitions (half:2) x (re/im:2)
        dst = bass.AP(tensor=out.tensor,
                      offset=rowA * 2 * TPR + t0,
                      ap=[[4 * TPR, 2], [TPR, 2], [1, CH]])
        nc.sync.dma_start(out=dst, in_=ot)
        del state[f]

    def emit_layer(grp, k):
        for f in grp:
            stage_vb(f, k)
        for f in grp:
            stage_vsq(f, k)
        for f in grp:
            stage_va(f, k)
        for f in grp:
            stage_rs(f, k)
        for f in grp:
            stage_w(f, k)
        for f in grp:
            stage_prelu(f, k)
        for f in grp:
            stage_dense(f, k)
        if k in (1, 3, 5):
            for f in grp:
                stage_carry(f, k)

    # ---- main loop: groups of G chunks, tails overlapped with the next
    # group's first layer ----
    groups = [list(range(f0, min(f0 + G, nchunks)))
              for f0 in range(0, nchunks, G)]
    prev = None
    for grp in groups:
        for f in grp:
            stage_load(f)
        for f in grp:
            stage_win(f)
        emit_layer(grp, 0)
        if prev is not None:
            for f in prev:
                tail_opmm(f)
            for f in prev:
                tail_store(f)
        for k in range(1, NL):
            emit_layer(grp, k)
        prev = grp
    for f in prev:
        tail_opmm(f)
    for f in prev:
        tail_store(f)
    ctx.close()


def _host_pack(inputs):
    """Build the shared (replicated) packed-weight arrays."""
    w_in = np.asarray(inputs["w_in"], np.float32)
    dense_w = np.asarray(inputs["dense_w"], np.float32)
    w_out = np.asarray(inputs["w_out"], np.float32)
    ln_gamma = np.asarray(inputs["ln_gamma"], np.float32)
    ln_beta = np.asarray(inputs["ln_beta"], np.float32)
    alpha = np.asarray(inputs["alpha"], np.float32)
    b_in = np.asarray(inputs["b_in"], np.float32)
    dense_b = np.asarray(inputs["dense_b"], np.float32)

    C = np.eye(F, dtype=np.float32) - 1.0 / F   # centering projector

    win = np.zeros((16, 128), np.float32)
    winC = w_in @ C
    win[0:8, 0:64] = winC
    win[8:16, 64:128] = winC

    wd = np.zeros((128, NL * 128), np.float32)
    ones6 = np.zeros((128, NL * 128), np.float32)
    for l in range(NL):
        wdC = dense_w[l] @ C
        wd[0:64, l * 128: l * 128 + 64] = wdC
        wd[64:128, l * 128 + 64: l * 128 + 128] = wdC
        g2 = ln_gamma[l] ** 2                     # [F]
        blk = np.repeat((1.0 / (F * g2))[None, :], F, axis=0)  # [F_in, F_out]
        ones6[0:64, l * 128: l * 128 + 64] = blk
        ones6[64:128, l * 128 + 64: l * 128 + 128] = blk

    wtail = np.zeros((128, 4), np.float32)
    wtail[0:64, 0:2] = w_out
    wtail[64:128, 2:4] = w_out

    s = w_out.sum(axis=0)                         # [2]
    wbar_in = w_in.mean(axis=1)                   # [8]
    fext = np.zeros((16, 4), np.float32)
    fext[0:8, 0:2] = np.outer(wbar_in, s)
    fext[8:16, 2:4] = np.outer(wbar_in, s)
    pext = np.zeros((128, 12), np.float32)
    for j, l in enumerate((1, 3, 5)):
        wbar = dense_w[l].mean(axis=1)            # [F]
        pext[0:64, 4 * j: 4 * j + 2] = np.outer(wbar, s)
        pext[64:128, 4 * j + 2: 4 * j + 4] = np.outer(wbar, s)

    # biases (centered-carry bookkeeping)
    bc_in = C @ b_in
    bc = [C @ dense_b[l] for l in range(NL)]
    vb_bias = {0: bc_in, 1: bc[0], 3: bc[2], 5: bc[4]}

    percol = np.zeros((128, NPC), np.float32)
    for k, v in vb_bias.items():
        percol[:, BR + k] = np.tile(v, 2)
    for k in (1, 3, 5):
        percol[:, BC + k] = np.tile(bc[k], 2)
    for k in range(NL):
        g = ln_gamma[k]
        percol[:, EG + k] = np.tile(EPS / (g * g), 2)
        percol[:, BE + k] = np.tile(ln_beta[k], 2)
        percol[:, AL + k] = np.tile(alpha[k], 2)
        percol[:, AB + k] = np.tile(alpha[k] * ln_beta[k], 2)

    # constant part of the final mean correction, folded into b_out
    m_const = b_in.mean() + dense_b[1].mean() + dense_b[3].mean() \
        + dense_b[5].mean()
    b_out_eff = np.asarray(inputs["b_out"], np.float32) + m_const * s

    bf_np = mybir.dt.np(BF)
    shared = dict(wd=wd, ones6=ones6, win=win, wtail=wtail, fext=fext,
                  pext=pext)
    shared = {k: np.ascontiguousarray(v.astype(bf_np))
              for k, v in shared.items()}
    shared["percol"] = percol
    return shared, b_out_eff


def _prep_x(xr, xi, dtype):
    """Pack [4, 2, TPR+3]: rows x (re/im) x left-padded-3 samples, bf16."""
    xr = np.pad(np.asarray(xr, np.float32), ((0, 0), (3, 0)))
    xi = np.pad(np.asarray(xi, np.float32), ((0, 0), (3, 0)))
    return np.ascontiguousarray(np.stack([xr, xi], axis=1).astype(dtype))


def build_program(tokens_per_row):
    """Build the full Bass/Tile program for one core's shard."""
    nc = bacc.Bacc("TRN2")
    ins = {}
    shapes = dict(wd=(128, NL * 128), ones6=(128, NL * 128), win=(16, 128),
                  wtail=(128, 4), fext=(16, 4), pext=(128, 12),
                  percol=(128, NPC))
    for name, shp in shapes.items():
        dt = F32 if name == "percol" else BF
        ins[name] = nc.dram_tensor(name, list(shp), dt,
                                   kind="ExternalInput").ap()
    ins["xp"] = nc.dram_tensor("xp", [4, 2, tokens_per_row + 3], BF,
                               kind="ExternalInput").ap()
    outs = {"out": nc.dram_tensor("out", [4, 2, tokens_per_row],
                                  F32, kind="ExternalOutput").ap()}
    with tile.TileContext(nc) as tc:
        build_kernel(tc, outs, ins, tokens_per_row)
    nc.compile()
    return nc


def _run(inputs, trace=False):
    from concourse.bass_utils import run_bass_kernel_spmd

    x_real = np.asarray(inputs["x_real"], np.float32)
    x_imag = np.asarray(inputs["x_imag"], np.float32)
    B, N = x_real.shape
    n_cores = 8
    rows_per_core = B // n_cores

    shared, b_out_eff = _host_pack(inputs)
    nc = build_program(N)
    bf_np = mybir.dt.np(BF)

    in_maps = []
    for c in range(n_cores):
        m = dict(shared)
        sl = slice(c * rows_per_core, (c + 1) * rows_per_core)
        m["xp"] = _prep_x(x_real[sl], x_imag[sl], bf_np)
        in_maps.append(m)

    res = run_bass_kernel_spmd(nc, in_maps, core_ids=list(range(n_cores)),
                               trace=trace)
    outs_np = [r["out"] for r in res.results]
    full = np.concatenate(outs_np, axis=0)          # [B, 2, N]
    re = full[:, 0, :] + b_out_eff[0] + x_real
    im = full[:, 1, :] + b_out_eff[1] + x_imag
    return (re + 1j * im).astype(np.complex64), res


def kernel(**inputs):
    return _run(inputs, trace=False)[0]
